# revision 27
# baseline (speedup 1.0000x reference)
"""Trainium2 Bass kernel for nn_Attention_31490700214694 (sparse_attention).

v3 design (per core = one batch x one channel-half, 8 cores total):

  Stage A (fused qkv 1x1 conv + 3x3 depthwise), two channel-group passes:
    pass1: channels [0:128); pass2: channels [128:256) + [256:288)
    per 16-row band: DMA x rows [r0-1, r0+17) -> qkv matmul (K=192 as
    128+64, PSUM chunks of 512 = 2 rows) -> ACT copies PSUM->SBUF into a
    258-column padded band tile (2 zero pad cols kill all w-wrap taps;
    halo rows at image edges are zeroed in the x tile so qkv==0 there)
    -> depthwise 3x3 as either TensorE diag-matmuls or DVE STT chains
    (per-unit assignment table) -> DMA band [ch, 16x256] to qkv_dw DRAM.
    The 32-channel remainder group is packed 4 bands per 128-partition
    tile (matmul writes PSUM at partition offset 32*(b%4)).

  Attention (2x2 windows on the raw-reshape aliased view
  [128 h', 256 w', 576 c] of the core's flat qkv_dw): per iteration
  NG=2 h'-row pairs; j=window-column rides partitions.  QK products on
  DVE (dense innermost d), d-reduction on GpSimd, exp WITHOUT
  d-expansion on ACT, softmax normalization folded in before a single
  d-expansion, AV products + j-sum, DMA out.  Iterations over channels
  [0:126) are emitted interleaved into pass2 so they overlap it.

  Proj 1x1 (192x96 partial) -> bf16 partial sums to DRAM; host sums the
  two halves per batch in f32 and un-shuffles.

Dependency tracking is AP-range based (verified), so DRAM round trips
pipeline; engine queues are in-order, so emission order = schedule.
"""

import os
import sys

import numpy as np

sys.path.insert(0, "/opt/trn_rl_repo")

def _install_ntff_hook():
    """Provide antenv.axon_hooks (missing in this image) so that
    run_bass_kernel_spmd(trace=True) can capture NTFF profiles."""
    import types
    import ctypes
    import contextlib

    if "antenv.axon_hooks" in sys.modules:
        return
    so_path = os.environ.get("PJRT_LIBRARY_PATH", "/opt/axon/libaxon_pjrt.so")
    try:
        lib = ctypes.CDLL(so_path)
    except OSError:
        return
    if not hasattr(lib, "axon_start_nrt_profile"):
        return
    lib.axon_start_nrt_profile.argtypes = [
        ctypes.POINTER(ctypes.c_int64), ctypes.c_size_t]
    lib.axon_start_nrt_profile.restype = ctypes.c_int64
    lib.axon_stop_nrt_profile.argtypes = [ctypes.c_char_p]
    lib.axon_stop_nrt_profile.restype = ctypes.c_int64

    @contextlib.contextmanager
    def _hook(output_dir, device_ids):
        import jax
        jax.devices()
        if device_ids:
            ids = (ctypes.c_int64 * len(device_ids))(*device_ids)
            rc = lib.axon_start_nrt_profile(ids, len(device_ids))
        else:
            rc = lib.axon_start_nrt_profile(None, 0)
        if rc != 0:
            raise RuntimeError(f"axon_start_nrt_profile rc={rc}")
        try:
            yield
        finally:
            n = lib.axon_stop_nrt_profile(str(output_dir).encode())
            if n < 0:
                raise RuntimeError(f"axon_stop_nrt_profile rc={n}")

    mod = types.ModuleType("antenv.axon_hooks")
    mod.get_axon_ntff_profile_hook = lambda: _hook
    mod.set_axon_ntff_profile_hook = lambda h: None
    sys.modules["antenv.axon_hooks"] = mod
    import antenv
    antenv.axon_hooks = mod


_install_ntff_hook()

import concourse.bass as bass
import concourse.tile as tile
from concourse import bacc, mybir
import concourse.bass_utils as _bu
from concourse.bass_utils import run_bass_kernel_spmd

# Skip the remote artifact upload in the profile path (no bucket here).
_bu.upload_artifacts = lambda tmpdir: tmpdir

F32 = mybir.dt.float32
BF16 = mybir.dt.bfloat16

C_IN = 192          # input channels (dim)
C_QKV = 288         # qkv channels per core (half of 576)
C_ATTN = 96         # attn output channels per core (half of 192)
NPX = 65536         # pixels per image
CTOK = 576          # channels per token in the aliased view
SCALE = 8 ** (-0.5)

RB = 16             # band rows
NBAND = 256 // RB   # 16 bands
RIN = RB + 2        # input rows incl halo
WPAD = 258          # padded row width
NDWIN = RIN * WPAD  # band tile free size
NACC = RB * WPAD    # dw output (padded) free size
NG = 2              # h'-row pairs per attention iteration
NITER = 64 // NG    # attention iterations

Add = mybir.AluOpType.add
Mult = mybir.AluOpType.mult
AX = mybir.AxisListType.X
ExpF = mybir.ActivationFunctionType.Exp
CopyF = mybir.ActivationFunctionType.Copy

# -------- tuning knobs --------
# depthwise engine per (pass, group, band-or-pack): 't' tensor, 'v' DVE,
# 'g' gpsimd. pass1 = group0 (16 bands); pass2 = group1 (16 bands) +
# group2 (4 packs).
DW_G0 = ['v'] * 10 + ['t'] * 6          # 16 bands of pass1
DW_G1 = ['t'] * 12 + ['v'] * 4          # 16 bands of pass2
DW_G2 = ['v', 't', 't', 'v']            # 4 packs (every 4 bands)
ATTN_REDUCE_ENG = 'v'                   # QK d-reduction: 'v' (DVE) or
                                        # 'g' = GpSimd add-tree
ATTN_EXPAND_ENG = 's'                   # EEn d-expansion: 's'(ACT), 'v', or
                                        # '' = no expansion (bcast read in AV)
ATTN_AVMULT_ENG = 'v'                   # AV product: 'g' or 'v'
PROJ_COPY_ENG = 's'                     # proj PSUM->SBUF: 's' or 'v'
                                        # (gpsimd cannot access PSUM)

_CACHE = {}


def _shuffle_perm(block=4):
    """src pixel index for each output pixel of shuffle_data (per channel)."""
    h = w = 256
    idx = np.arange(h * w).reshape(1, 1, h, w)
    x = np.transpose(idx, (0, 2, 3, 1)).reshape(1, h * w, 1)
    x = x.reshape(1, block, h // block, block, w // block, 1)
    x = np.transpose(x, (0, 2, 4, 1, 3, 5)).reshape(1, h * w, 1)
    return x.reshape(h * w).copy()


def _shuffle_back_perm(block=4):
    h = w = 256
    idx = np.arange(h * w).reshape(1, 1, h, w)
    x = np.transpose(idx, (0, 2, 3, 1)).reshape(1, h * w, 1)
    x = x.reshape(1, h // block, w // block, block, block, 1)
    x = np.transpose(x, (0, 3, 1, 4, 2, 5)).reshape(1, h * w, 1)
    return x.reshape(h * w).copy()


def _eng(nc, e):
    return {'v': nc.vector, 'g': nc.gpsimd, 's': nc.scalar,
            't': nc.tensor, 'y': nc.sync}[e]


def _dw_compute(nc, tc, pools, dwin, gi, eng, wdwg, wdiag_t, ppool):
    """3x3 depthwise on one band tile dwin [128, RIN*WPAD] -> returns an
    SBUF tile [128, ...] holding the RB*256 (or RB*WPAD padded) output,
    plus a flag whether it is padded (258-stride) layout."""
    # dwin row layout: col 0 = left pad, cols 1..256 = image cols 0..255,
    # col 257 = right pad (pads zero).  acc: image col c at acc col c
    # (even base so DVE 2x mode applies; odd-offset taps read a one-elem
    # shifted copy SC built on gpsimd).
    acc = pools['dwacc'].tile([128, NACC], BF16, tag="dwacc")
    if eng == 't':
        av = acc[:].rearrange("p (r w) -> p r w", w=WPAD)
        nch = RB * 256 // 512
        for c in range(nch):
            r0 = 2 * c
            ps = ppool.tile([128, 512], F32, tag=f"dw{c % 2}")
            t9 = 0
            for dh in (-1, 0, 1):
                for dwc in (-1, 0, 1):
                    mv = dwin[:].rearrange("p (r w) -> p r w", w=WPAD)[
                        :, r0 + 1 + dh: r0 + 3 + dh, 1 + dwc: 257 + dwc]
                    nc.tensor.matmul(
                        ps[:], wdiag_t[:, bass.ts(t9, 128)], mv,
                        start=(t9 == 0), stop=(t9 == 8))
                    t9 += 1
            nc.scalar.copy(av[:, r0:r0 + 2, 0:256],
                           ps[:].rearrange("p (r w) -> p r w", w=256))
    else:
        e = _eng(nc, eng)
        SC = pools['dwacc'].tile([128, NDWIN], BF16, tag="dwsc")
        nc.gpsimd.tensor_copy(SC[:, 0:NDWIN - 1], dwin[:, 1:NDWIN])
        n2 = NACC - 2
        first = True
        for dh in (-1, 0, 1):
            for dwc in (-1, 0, 1):
                t9 = (dh + 1) * 3 + (dwc + 1)
                d = WPAD * (1 + dh) + dwc + 1
                if d % 2:
                    sap = SC[:, d - 1:d - 1 + n2]
                else:
                    sap = dwin[:, d:d + n2]
                wt = wdwg[:, 9 * gi + t9: 9 * gi + t9 + 1]
                if first:
                    e.tensor_scalar_mul(acc[:, 0:n2], sap, wt)
                    first = False
                else:
                    e.scalar_tensor_tensor(
                        acc[:, 0:n2], sap, wt, acc[:, 0:n2],
                        op0=Mult, op1=Add)
    return acc


def _emit_band(nc, tc, pools, ppool, xs, wq0, wq1, qkv_dw, wdwg, wdiag,
               b, groups, dw_eng_of):
    """One 16-row band of stage A for the given channel groups."""
    r_top = RB * b - 1
    lo = max(0, r_top)
    hi = min(256, r_top + RIN)
    x0 = pools['x'].tile([128, RIN * 256], BF16, tag="x0")
    x1 = pools['x'].tile([64, RIN * 256], BF16, tag="x1")
    d0 = (lo - r_top) * 256
    nc.sync.dma_start(x0[:, d0:d0 + (hi - lo) * 256],
                      xs[0:128, lo * 256:hi * 256])
    nc.sync.dma_start(x1[:, d0:d0 + (hi - lo) * 256],
                      xs[128:192, lo * 256:hi * 256])
    if lo > r_top:
        nc.vector.memset(x0[:, 0:256], 0.0)
        nc.vector.memset(x1[:, 0:256], 0.0)
    if hi < r_top + RIN:
        nc.vector.memset(x0[:, (RIN - 1) * 256:], 0.0)
        nc.vector.memset(x1[:, (RIN - 1) * 256:], 0.0)

    for gi in groups:
        m0 = 128 * gi
        mm = min(128, C_QKV - m0)
        if gi < 2:
            dwin = pools['dwin'].tile([128, NDWIN], BF16, tag="dwinA")
            pslc = slice(0, mm)
        else:
            if b % 4 == 0:
                g2tile = pools['dwin'].tile(
                    [128, NDWIN], BF16, tag="dwin2")
                pools['g2tile'] = g2tile
            dwin = pools['g2tile']
            pslc = slice(32 * (b % 4), 32 * (b % 4) + 32)
        # zero the two pad columns (left col 0, right col 257)
        dv = dwin[pslc].rearrange("p (r w) -> p r w", w=WPAD)
        nc.vector.memset(dv[:, :, 0:1], 0.0)
        nc.vector.memset(dv[:, :, 257:258], 0.0)
        # qkv matmul in chunks of 512 px (2 rows), ACT-copy into dwin
        for n in range(RIN * 256 // 512):
            ps = ppool.tile([128, 512], F32, tag=f"qk{n % 2}")
            po = ps[pslc] if gi == 2 else ps[:mm]
            tp = {"tile_position": (0, pslc.start)} if gi == 2 else {}
            nc.tensor.matmul(po, wq0[:, m0:m0 + mm],
                             x0[:, bass.ts(n, 512)], start=True, stop=False,
                             **tp)
            nc.tensor.matmul(po, wq1[:, m0:m0 + mm],
                             x1[:, bass.ts(n, 512)], start=False, stop=True,
                             **tp)
            nc.scalar.copy(dv[:, 2 * n:2 * n + 2, 1:257],
                           po.rearrange("p (r w) -> p r w", w=256))
        # depthwise + store
        if gi < 2:
            eng = dw_eng_of(gi, b)
            out = _dw_compute(nc, tc, pools, dwin, gi, eng,
                              wdwg, wdiag[gi], ppool)
            src = out[:mm].rearrange("p (r w) -> p r w", w=WPAD)[:, :, 0:256]
            nc.sync.dma_start(
                qkv_dw[m0:m0 + mm, RB * 256 * b: RB * 256 * (b + 1)], src)
        elif b % 4 == 3:
            eng = dw_eng_of(gi, b // 4)
            out = _dw_compute(nc, tc, pools, dwin, gi, eng,
                              wdwg, wdiag[gi], ppool)
            for k in range(4):
                bb = b - 3 + k
                src = out[32 * k:32 * k + 32].rearrange(
                    "p (r w) -> p r w", w=WPAD)[:, :, 0:256]
                nc.sync.dma_start(
                    qkv_dw[256:288, RB * 256 * bb: RB * 256 * (bb + 1)], src)


def _emit_attn_iter(nc, tc, pools, qv, av, it):
    """One attention iteration: NG=2 h'-row pairs (4 h'-rows)."""
    T = pools['at'].tile([128, NG * 4 * CTOK], BF16, tag="T")
    src = qv[4 * it:4 * it + 4].rearrange(
        "(g dh) (j dw) c -> j g dh dw c", dh=2, dw=2)
    nc.sync.dma_start(
        T[:].rearrange("p (g dh dw c) -> p g dh dw c", g=NG, dh=2, dw=2), src)
    t4 = T[:].rearrange("p (g t c) -> p g t c", g=NG, t=4)
    q = t4[:, :, :, 0:192]
    k = t4[:, :, :, 192:384]
    v = t4[:, :, :, 384:576]

    # QK products: per (g, i) ops, k fully dense, q_i broadcast over j
    P = pools['big'].tile([128, NG * 3072], BF16, tag="big")
    pg = P[:].rearrange("p (g i j hd) -> p g i j hd", g=NG, i=4, j=4)
    for g in range(NG):
        for i in range(4):
            q_b = q[:, g, i].unsqueeze(1).broadcast_to([128, 4, 192])
            nc.vector.tensor_tensor(pg[:, g, i], q_b, k[:, g], op=Mult)

    # logits: reduce over d (24)
    L = pools['sm'].tile([128, NG * 128], F32, tag="L")
    if ATTN_REDUCE_ENG == 'v':
        nc.vector.tensor_reduce(
            L[:], P[:].rearrange("p (s d) -> p s d", d=24), axis=AX, op=Add)
    else:
        # GpSimd has no free-axis reduce; use a tensor_tensor add tree
        ps = P[:].rearrange("p (s d) -> p s d", d=24)
        B8 = pools['sm'].tile([128, NG * 128 * 8], BF16, tag="B8")
        b8 = B8[:].rearrange("p (s d) -> p s d", d=8)
        nc.gpsimd.tensor_tensor(b8, ps[:, :, 0:8], ps[:, :, 8:16], op=Add)
        nc.gpsimd.tensor_tensor(b8, b8, ps[:, :, 16:24], op=Add)
        B2 = pools['sm'].tile([128, NG * 128 * 2], BF16, tag="B2")
        b2 = B2[:].rearrange("p (s d) -> p s d", d=2)
        nc.gpsimd.tensor_tensor(
            b2, b8[:, :, 0:2],
            b8[:, :, 2:4], op=Add)
        nc.gpsimd.tensor_tensor(
            b2, b2, b8[:, :, 4:6], op=Add)
        nc.gpsimd.tensor_tensor(
            b2, b2, b8[:, :, 6:8], op=Add)
        nc.gpsimd.tensor_tensor(
            L[:].rearrange("p (s d) -> p s d", d=1),
            b2[:, :, 0:1], b2[:, :, 1:2], op=Add)

    # exp (no expansion), then fold in 1/S.  (g i) merges to one dim.
    EE = pools['sm'].tile([128, NG * 128], BF16, tag="EE")
    nc.scalar.activation(EE[:], L[:], ExpF, scale=float(SCALE))
    S = pools['sm'].tile([128, NG * 32], F32, tag="S")
    ee3 = EE[:].rearrange("p (gi j h) -> p gi j h", j=4, h=8)
    nc.vector.tensor_reduce(
        S[:].rearrange("p (gi h) -> p gi h", h=8),
        ee3.rearrange("p gi j h -> p gi h j"), axis=AX, op=Add)
    R = pools['sm'].tile([128, NG * 32], F32, tag="R")
    nc.vector.reciprocal(R[:], S[:])
    EN = pools['sm'].tile([128, NG * 128], BF16, tag="EN")
    r_b = R[:].rearrange("p (gi h) -> p gi h", h=8).unsqueeze(
        2).broadcast_to([128, NG * 4, 4, 8])
    nc.gpsimd.tensor_tensor(
        EN[:].rearrange("p (gi j h) -> p gi j h", j=4, h=8),
        ee3, r_b, op=Mult)

    # AV products: expand EN over d, then dense mult with v (bcast over i,
    # per-g since (g i) can't merge on the v side)
    P2 = pools['big'].tile([128, NG * 3072], BF16, tag="big")
    pg2 = P2[:].rearrange("p (g i j hd) -> p g i j hd", g=NG, i=4, j=4)
    EX = pools['big'].tile([128, NG * 3072], BF16, tag="big")
    exv = EX[:].rearrange("p (s d) -> p s d", d=24)
    enb = EN[:].unsqueeze(2).broadcast_to([128, NG * 128, 24])
    if ATTN_EXPAND_ENG == 's':
        nc.scalar.copy(exv, enb)
    else:
        _eng(nc, ATTN_EXPAND_ENG).tensor_copy(exv, enb)
    exg = EX[:].rearrange("p (g i j hd) -> p g i j hd", g=NG, i=4, j=4)
    for g in range(NG):
        for i in range(4):
            _eng(nc, ATTN_AVMULT_ENG).tensor_tensor(
                pg2[:, g, i], exg[:, g, i], v[:, g], op=Mult)

    # sum over j ((g i) merges)
    A = pools['at'].tile([128, NG * 1536], BF16, tag="A")
    a4 = A[:].rearrange("p (gi u hd) -> p gi u hd", u=2, hd=192)
    p24 = P2[:].rearrange("p (gi j hd) -> p gi j hd", j=4, hd=192)
    nc.vector.tensor_tensor(a4, p24[:, :, 0:2, :], p24[:, :, 2:4, :], op=Add)
    U = pools['at'].tile([128, NG * 768], BF16, tag="U")
    u3 = U[:].rearrange("p (gi hd) -> p gi hd", hd=192)
    nc.vector.tensor_tensor(u3, a4[:, :, 0], a4[:, :, 1], op=Add)

    dst = av[4 * it:4 * it + 4].rearrange(
        "(g dh) (j dw) c -> j g dh dw c", dh=2, dw=2)
    nc.scalar.dma_start(
        dst, U[:].rearrange("p (g dh dw c) -> p g dh dw c", g=NG, dh=2, dw=2))


def _emit_proj(nc, tc, pools, ppool, attn_dram, wp, out_dram):
    NT = 1024
    for n in range(NPX // NT):
        xt = pools['pj'].tile([C_ATTN, NT], BF16, tag="pjx")
        nc.sync.dma_start(xt[:], attn_dram[:, bass.ts(n, NT)])
        for mi, (m0, m1) in enumerate([(0, 128), (128, 192)]):
            mm = m1 - m0
            ot = pools['pj'].tile([128, NT], BF16, tag=f"pjo{mi}")
            for s in range(NT // 512):
                ps = ppool.tile([128, 512], F32, tag=f"pj{s % 2}")
                nc.tensor.matmul(ps[:mm], wp[:, m0:m1],
                                 xt[:, bass.ts(s, 512)], start=True, stop=True)
                if PROJ_COPY_ENG == 's':
                    nc.scalar.copy(ot[:mm, bass.ts(s, 512)], ps[:mm])
                else:
                    nc.vector.tensor_copy(ot[:mm, bass.ts(s, 512)], ps[:mm])
            nc.scalar.dma_start(out_dram[m0:m1, bass.ts(n, NT)], ot[:mm])


def _build():
    if "nc" in _CACHE:
        return _CACHE["nc"]
    nc = bacc.Bacc("TRN2", target_bir_lowering=False, debug=False,
                   num_devices=8)
    xs = nc.dram_tensor("xs", [C_IN, NPX], BF16, kind="ExternalInput").ap()
    wqkvT = nc.dram_tensor("wqkvT", [C_IN, C_QKV], BF16,
                           kind="ExternalInput").ap()
    wdwg_d = nc.dram_tensor("wdwg", [128, 27], F32, kind="ExternalInput").ap()
    wdiag_d = nc.dram_tensor("wdiag", [3, 9, 128, 128], BF16,
                             kind="ExternalInput").ap()
    wprojT = nc.dram_tensor("wprojT", [C_ATTN, 192], BF16,
                            kind="ExternalInput").ap()
    out = nc.dram_tensor("out", [192, NPX], BF16, kind="ExternalOutput").ap()

    qkv_dw = nc.dram_tensor("qkv_dw_buf", [C_QKV, NPX], BF16).ap()
    attn_dram = nc.dram_tensor("attn_buf", [C_ATTN, NPX], BF16).ap()

    qv = qkv_dw.rearrange("c p -> (c p)").rearrange(
        "(hh ww cc) -> hh ww cc", ww=256, cc=CTOK)
    av = attn_dram.rearrange("c p -> (c p)").rearrange(
        "(hh ww cc) -> hh ww cc", ww=256, cc=192)

    from contextlib import ExitStack
    with tile.TileContext(nc) as tc:
        with ExitStack() as ctx:
            pools = {}
            ppool = ctx.enter_context(
                tc.tile_pool(name="psum", bufs=1, space="PSUM"))
            pools['x'] = ctx.enter_context(tc.tile_pool(name="x", bufs=2))
            pools['dwin'] = ctx.enter_context(
                tc.tile_pool(name="dwin", bufs=2))
            pools['dwacc'] = ctx.enter_context(
                tc.tile_pool(name="dwacc", bufs=2))
            pools['w'] = ctx.enter_context(tc.tile_pool(name="w", bufs=1))
            pools['at'] = ctx.enter_context(tc.tile_pool(name="at", bufs=2))
            pools['big'] = ctx.enter_context(tc.tile_pool(name="big", bufs=3))
            pools['sm'] = ctx.enter_context(tc.tile_pool(name="sm", bufs=2))
            pools['pj'] = ctx.enter_context(tc.tile_pool(name="pj", bufs=2))

            # weights
            wq0 = pools['w'].tile([128, C_QKV], BF16, tag="wq0")
            wq1 = pools['w'].tile([64, C_QKV], BF16, tag="wq1")
            nc.sync.dma_start(wq0[:], wqkvT[0:128, :])
            nc.sync.dma_start(wq1[:], wqkvT[128:192, :])
            wdwg = pools['w'].tile([128, 27], F32, tag="wdwg")
            nc.sync.dma_start(wdwg[:], wdwg_d)
            wdiag = []
            for gi in range(3):
                wt = pools['w'].tile([128, 9 * 128], BF16, tag=f"wdiag{gi}")
                nc.sync.dma_start(
                    wt[:].rearrange("p (t m) -> p t m", t=9),
                    wdiag_d[gi].rearrange("t k m -> k t m"))
                wdiag.append(wt)
            wp = pools['w'].tile([C_ATTN, 192], BF16, tag="wp")
            nc.sync.dma_start(wp[:], wprojT)

            # pass 1: group 0
            for b in range(NBAND):
                _emit_band(nc, tc, pools, ppool, xs, wq0, wq1, qkv_dw,
                           wdwg, wdiag, b, [0],
                           lambda gi, i: DW_G0[i])
            # pass 2: groups 1+2, with early attention iters interleaved
            early = [i for i in range(NITER) if 4.5 * (2 * i + 2) <= 128.0]
            ei = 0
            for b in range(NBAND):
                _emit_band(nc, tc, pools, ppool, xs, wq0, wq1, qkv_dw,
                           wdwg, wdiag, b, [1, 2],
                           lambda gi, i: DW_G1[i] if gi == 1 else DW_G2[i])
                want = (b + 1) * len(early) // NBAND
                while ei < want:
                    _emit_attn_iter(nc, tc, pools, qv, av, early[ei])
                    ei += 1
            for it in range(len(early), NITER):
                _emit_attn_iter(nc, tc, pools, qv, av, it)
            _emit_proj(nc, tc, pools, ppool, attn_dram, wp, out)
    nc.compile()
    _CACHE["nc"] = nc
    return nc


def kernel(x, w_qkv, w_dw, w_proj, shuffle):
    import ml_dtypes
    bf = ml_dtypes.bfloat16
    x = np.asarray(x, dtype=np.float32)
    w_qkv = np.asarray(w_qkv, dtype=np.float32)
    w_dw = np.asarray(w_dw, dtype=np.float32)
    w_proj = np.asarray(w_proj, dtype=np.float32)
    do_shuffle = bool(int(np.asarray(shuffle)))

    B = x.shape[0]
    xf = x.reshape(B, C_IN, NPX)
    if do_shuffle:
        xf = xf[:, :, _shuffle_perm()]

    wq = w_qkv[:, :, 0, 0]                      # [576, 192]
    wqT = np.ascontiguousarray(wq.T)            # [192, 576]
    wdw_f = w_dw[:, 0].reshape(576, 9)          # [576, 9]
    wpj = w_proj[:, :, 0, 0]                    # [192, 192]

    in_maps = []
    for b in range(B):
        for s in range(2):
            wdw_h = wdw_f[s * C_QKV:(s + 1) * C_QKV]      # [288, 9]
            # group tap weights [128, 27]: group gi cols 9gi..9gi+9,
            # partition p -> channel 128*gi+p (gi<2) or 256+p%32
            wdwg = np.zeros((128, 27), dtype=np.float32)
            wdwg[:, 0:9] = wdw_h[0:128]
            wdwg[:, 9:18] = wdw_h[128:256]
            wdwg[:, 18:27] = wdw_h[256 + (np.arange(128) % 32)]
            wdiag = np.zeros((3, 9, 128, 128), dtype=bf)
            for gi in range(3):
                ch = (np.arange(128) + 128 * gi if gi < 2
                      else 256 + np.arange(128) % 32)
                for t in range(9):
                    wdiag[gi, t][np.arange(128), np.arange(128)] = \
                        wdw_h[ch, t].astype(bf)
            in_maps.append({
                "xs": np.ascontiguousarray(xf[b]).astype(bf),
                "wqkvT": np.ascontiguousarray(
                    wqT[:, s * C_QKV:(s + 1) * C_QKV]).astype(bf),
                "wdwg": wdwg,
                "wdiag": wdiag,
                "wprojT": np.ascontiguousarray(
                    wpj[:, s * C_ATTN:(s + 1) * C_ATTN].T).astype(bf),
            })

    nc = _build()
    res = run_bass_kernel_spmd(nc, in_maps, core_ids=list(range(8)),
                               trace=bool(int(os.environ.get("KERNEL_TRACE", "0"))))
    _CACHE["last_results"] = res

    outs = [res.results[i]["out"] for i in range(8)]
    of = np.stack([outs[2 * b].astype(np.float32) + outs[2 * b + 1].astype(np.float32)
                   for b in range(B)])
    if do_shuffle:
        of = of[:, :, _shuffle_back_perm()]
    return of.reshape(B, 192, 256, 256).astype(np.float32)


# revision 41
# speedup vs baseline: 1.4632x; 1.4632x over previous
"""Trainium2 Bass kernel for nn_Attention_31490700214694 (sparse_attention).

v3 design (per core = one batch x one channel-half, 8 cores total):

  Stage A (fused qkv 1x1 conv + 3x3 depthwise), two channel-group passes:
    pass1: channels [0:128); pass2: channels [128:256) + [256:288)
    per 16-row band: DMA x rows [r0-1, r0+17) -> qkv matmul (K=192 as
    128+64, PSUM chunks of 512 = 2 rows) -> ACT copies PSUM->SBUF into a
    258-column padded band tile (2 zero pad cols kill all w-wrap taps;
    halo rows at image edges are zeroed in the x tile so qkv==0 there)
    -> depthwise 3x3 as either TensorE diag-matmuls or DVE STT chains
    (per-unit assignment table) -> DMA band [ch, 16x256] to qkv_dw DRAM.
    The 32-channel remainder group is packed 4 bands per 128-partition
    tile (matmul writes PSUM at partition offset 32*(b%4)).

  Attention (2x2 windows on the raw-reshape aliased view
  [128 h', 256 w', 576 c] of the core's flat qkv_dw): per iteration
  NG=2 h'-row pairs; j=window-column rides partitions.  QK products on
  DVE (dense innermost d), d-reduction on GpSimd, exp WITHOUT
  d-expansion on ACT, softmax normalization folded in before a single
  d-expansion, AV products + j-sum, DMA out.  Iterations over channels
  [0:126) are emitted interleaved into pass2 so they overlap it.

  Proj 1x1 (192x96 partial) -> bf16 partial sums to DRAM; host sums the
  two halves per batch in f32 and un-shuffles.

Dependency tracking is AP-range based (verified), so DRAM round trips
pipeline; engine queues are in-order, so emission order = schedule.
"""

import os
import sys

import numpy as np

sys.path.insert(0, "/opt/trn_rl_repo")

def _install_ntff_hook():
    """Provide antenv.axon_hooks (missing in this image) so that
    run_bass_kernel_spmd(trace=True) can capture NTFF profiles."""
    import types
    import ctypes
    import contextlib

    if "antenv.axon_hooks" in sys.modules:
        return
    so_path = os.environ.get("PJRT_LIBRARY_PATH", "/opt/axon/libaxon_pjrt.so")
    try:
        lib = ctypes.CDLL(so_path)
    except OSError:
        return
    if not hasattr(lib, "axon_start_nrt_profile"):
        return
    lib.axon_start_nrt_profile.argtypes = [
        ctypes.POINTER(ctypes.c_int64), ctypes.c_size_t]
    lib.axon_start_nrt_profile.restype = ctypes.c_int64
    lib.axon_stop_nrt_profile.argtypes = [ctypes.c_char_p]
    lib.axon_stop_nrt_profile.restype = ctypes.c_int64

    @contextlib.contextmanager
    def _hook(output_dir, device_ids):
        import jax
        jax.devices()
        if device_ids:
            ids = (ctypes.c_int64 * len(device_ids))(*device_ids)
            rc = lib.axon_start_nrt_profile(ids, len(device_ids))
        else:
            rc = lib.axon_start_nrt_profile(None, 0)
        if rc != 0:
            raise RuntimeError(f"axon_start_nrt_profile rc={rc}")
        try:
            yield
        finally:
            n = lib.axon_stop_nrt_profile(str(output_dir).encode())
            if n < 0:
                raise RuntimeError(f"axon_stop_nrt_profile rc={n}")

    mod = types.ModuleType("antenv.axon_hooks")
    mod.get_axon_ntff_profile_hook = lambda: _hook
    mod.set_axon_ntff_profile_hook = lambda h: None
    sys.modules["antenv.axon_hooks"] = mod
    import antenv
    antenv.axon_hooks = mod


_install_ntff_hook()

import concourse.bass as bass
import concourse.tile as tile
from concourse import bacc, mybir
import concourse.bass_utils as _bu
from concourse.bass_utils import run_bass_kernel_spmd

# Skip the remote artifact upload in the profile path (no bucket here).
_bu.upload_artifacts = lambda tmpdir: tmpdir

# Enable the backend LDWEIGHTS elision pass (stationary reuse): each matmul
# otherwise pays a full stationary reload (~180ns at 512 cols).
if os.environ.get("KERNEL_LDW_OPT", "0") == "1":
    _orig_run_command = _bu.run_command

    def _run_command_ldw(cmd, *a, **kw):
        cmd = ["--enable-ldw-opt=true" if c == "--enable-ldw-opt=false" else c
               for c in cmd]
        return _orig_run_command(cmd, *a, **kw)

    _bu.run_command = _run_command_ldw

F32 = mybir.dt.float32
BF16 = mybir.dt.bfloat16

C_IN = 192          # input channels (dim)
C_QKV = 288         # qkv channels per core (half of 576)
C_ATTN = 96         # attn output channels per core (half of 192)
NPX = 65536         # pixels per image
CTOK = 576          # channels per token in the aliased view
SCALE = 8 ** (-0.5)

RB = 16             # band rows
NBAND = 256 // RB   # 16 bands
RIN = RB + 2        # input rows incl halo
WPAD = 258          # padded row width
NDWIN = RIN * WPAD  # band tile free size
NACC = RB * WPAD    # dw output (padded) free size
NG = 2              # h'-row pairs per attention iteration
NITER = 64 // NG    # attention iterations

Add = mybir.AluOpType.add
Mult = mybir.AluOpType.mult
AX = mybir.AxisListType.X
ExpF = mybir.ActivationFunctionType.Exp
CopyF = mybir.ActivationFunctionType.Copy

# -------- tuning knobs --------
# depthwise engine per (pass, group, band-or-pack): 't' tensor, 'v' DVE,
# 'g' gpsimd. pass1 = group0 (16 bands); pass2 = group1 (16 bands) +
# group2 (4 packs).
DW_G0 = ['v'] * 8 + ['t'] * 8           # 16 bands of pass1
DW_G1 = ['t'] * 10 + ['v'] * 6          # 16 bands of pass2
DW_G2 = ['v', 't', 't', 'v']            # 4 packs (every 4 bands)
ATTN_REDUCE_ENG = 'v'                   # QK d-reduction: 'v' (DVE) or
                                        # 'g' = GpSimd add-tree
ATTN_EXPAND_ENG = 's'                   # EEn d-expansion: 's'(ACT), 'v', or
                                        # '' = no expansion (bcast read in AV)
ATTN_AVMULT_ENG = 'v'                   # AV product: 'g' or 'v'
PROJ_COPY_ENG = 's'                     # proj PSUM->SBUF: 's' or 'v'
                                        # (gpsimd cannot access PSUM)

_CACHE = {}


def _shuffle_perm(block=4):
    """src pixel index for each output pixel of shuffle_data (per channel)."""
    h = w = 256
    idx = np.arange(h * w).reshape(1, 1, h, w)
    x = np.transpose(idx, (0, 2, 3, 1)).reshape(1, h * w, 1)
    x = x.reshape(1, block, h // block, block, w // block, 1)
    x = np.transpose(x, (0, 2, 4, 1, 3, 5)).reshape(1, h * w, 1)
    return x.reshape(h * w).copy()


def _shuffle_back_perm(block=4):
    h = w = 256
    idx = np.arange(h * w).reshape(1, 1, h, w)
    x = np.transpose(idx, (0, 2, 3, 1)).reshape(1, h * w, 1)
    x = x.reshape(1, h // block, w // block, block, block, 1)
    x = np.transpose(x, (0, 3, 1, 4, 2, 5)).reshape(1, h * w, 1)
    return x.reshape(h * w).copy()


def _eng(nc, e):
    return {'v': nc.vector, 'g': nc.gpsimd, 's': nc.scalar,
            't': nc.tensor, 'y': nc.sync}[e]


def _dw_compute(nc, tc, pools, dwin, gi, eng, wdwg, wdiag_t, ppool):
    """3x3 depthwise on one band tile dwin [128, RIN*WPAD] -> returns an
    SBUF tile [128, ...] holding the RB*256 (or RB*WPAD padded) output,
    plus a flag whether it is padded (258-stride) layout."""
    # dwin row layout: col 0 = left pad, cols 1..256 = image cols 0..255,
    # col 257 = right pad (pads zero).  acc: image col c at acc col c
    # (even base so DVE 2x mode applies; odd-offset taps read a one-elem
    # shifted copy SC built on gpsimd).
    acc = pools['dwacc'].tile([128, NACC], BF16, tag="dwacc")
    if eng == 't':
        av = acc[:].rearrange("p (r w) -> p r w", w=WPAD)
        nch = RB * 256 // 512
        for c in range(nch):
            r0 = 2 * c
            ps = ppool.tile([128, 512], F32, tag=f"dw{c % 2}")
            t9 = 0
            for dh in (-1, 0, 1):
                for dwc in (-1, 0, 1):
                    mv = dwin[:].rearrange("p (r w) -> p r w", w=WPAD)[
                        :, r0 + 1 + dh: r0 + 3 + dh, 1 + dwc: 257 + dwc]
                    nc.tensor.matmul(
                        ps[:], wdiag_t[:, bass.ts(t9, 128)], mv,
                        start=(t9 == 0), stop=(t9 == 8))
                    t9 += 1
            nc.scalar.copy(av[:, r0:r0 + 2, 0:256],
                           ps[:].rearrange("p (r w) -> p r w", w=256))
    else:
        # tensor_scalar runs 4x on DVE (bf16, SBUF) and tensor_tensor 2x;
        # scalar_tensor_tensor only has a 1x uop.  So: scale each tap with
        # tensor_scalar_mul into a temp, accumulate with tensor_tensor.
        e = _eng(nc, eng)
        n2 = NACC - 2
        y0 = pools['dwy'].tile([128, NACC], BF16, tag="dwy0")
        y1 = pools['dwy'].tile([128, NACC], BF16, tag="dwy1")
        taps = [(dh, dwc) for dh in (-1, 0, 1) for dwc in (-1, 0, 1)]

        def wslice(dh, dwc):
            t9 = (dh + 1) * 3 + (dwc + 1)
            return wdwg[:, 9 * gi + t9: 9 * gi + t9 + 1]

        def sap(dh, dwc):
            d = WPAD * (1 + dh) + dwc + 1
            return dwin[:, d:d + n2]

        e.tensor_scalar_mul(y0[:, 0:n2], sap(*taps[0]), wslice(*taps[0]))
        e.tensor_scalar_mul(y1[:, 0:n2], sap(*taps[1]), wslice(*taps[1]))
        e.tensor_tensor(acc[:, 0:n2], y0[:, 0:n2], y1[:, 0:n2], op=Add)
        for t in range(2, 9):
            yt = y0 if t % 2 == 0 else y1
            e.tensor_scalar_mul(yt[:, 0:n2], sap(*taps[t]), wslice(*taps[t]))
            e.tensor_tensor(acc[:, 0:n2], acc[:, 0:n2], yt[:, 0:n2], op=Add)
    return acc


def _emit_band(nc, tc, pools, ppool, xs, wq0, wq1, qkv_dw, wdwg, wdiag,
               b, groups, dw_eng_of):
    """One 16-row band of stage A for the given channel groups."""
    r_top = RB * b - 1
    lo = max(0, r_top)
    hi = min(256, r_top + RIN)
    x0 = pools['x'].tile([128, RIN * 256], BF16, tag="x0")
    x1 = pools['x'].tile([64, RIN * 256], BF16, tag="x1")
    d0 = (lo - r_top) * 256
    nc.sync.dma_start(x0[:, d0:d0 + (hi - lo) * 256],
                      xs[0:128, lo * 256:hi * 256])
    nc.sync.dma_start(x1[:, d0:d0 + (hi - lo) * 256],
                      xs[128:192, lo * 256:hi * 256])
    if lo > r_top:
        nc.vector.memset(x0[:, 0:256], 0.0)
        nc.vector.memset(x1[:, 0:256], 0.0)
    if hi < r_top + RIN:
        nc.vector.memset(x0[:, (RIN - 1) * 256:], 0.0)
        nc.vector.memset(x1[:, (RIN - 1) * 256:], 0.0)

    for gi in groups:
        m0 = 128 * gi
        mm = min(128, C_QKV - m0)
        if gi < 2:
            dwin = pools['dwin'].tile([128, NDWIN], BF16, tag="dwinA")
            pslc = slice(0, mm)
            nver = pools['nver_A'] = pools.get('nver_A', -1) + 1
        else:
            if b % 4 == 0:
                g2tile = pools['dwin'].tile(
                    [128, NDWIN], BF16, tag="dwin2")
                pools['g2tile'] = g2tile
                pools['nver_2'] = pools.get('nver_2', -1) + 1
            dwin = pools['g2tile']
            pslc = slice(32 * (b % 4), 32 * (b % 4) + 32)
            nver = pools['nver_2']
        # zero the two pad columns (left col 0, right col 257).  Pads are
        # never overwritten, so only the first use of each rotating pool
        # buffer needs the memset.
        dv = dwin[pslc].rearrange("p (r w) -> p r w", w=WPAD)
        if nver < 2:
            nc.vector.memset(dv[:, :, 0:1], 0.0)
            nc.vector.memset(dv[:, :, 257:258], 0.0)
        # qkv matmul into 2-bank PSUM chunks (4 rows = 1024 px), fewer
        # bigger ACT copies into dwin
        chunks = [(0, 4), (4, 8), (8, 12), (12, 16), (16, 18)]
        for ci, (r0, r1) in enumerate(chunks):
            nr = r1 - r0
            ps = ppool.tile([128, 1024], F32, tag=f"qk{ci % 2}")
            tp = {"tile_position": (0, pslc.start)} if gi == 2 else {}
            for h in range(nr // 2):
                po = ps[pslc, 512 * h:512 * h + 512] if gi == 2 \
                    else ps[:mm, 512 * h:512 * h + 512]
                px0 = (r0 + 2 * h) * 256
                nc.tensor.matmul(po, wq0[:, m0:m0 + mm],
                                 x0[:, px0:px0 + 512], start=True, stop=False,
                                 **tp)
                nc.tensor.matmul(po, wq1[:, m0:m0 + mm],
                                 x1[:, px0:px0 + 512], start=False, stop=True,
                                 **tp)
            pc = ps[pslc, 0:256 * nr] if gi == 2 else ps[:mm, 0:256 * nr]
            nc.scalar.copy(dv[:, r0:r1, 1:257],
                           pc.rearrange("p (r w) -> p r w", w=256))
        # depthwise + store
        if gi < 2:
            eng = dw_eng_of(gi, b)
            out = _dw_compute(nc, tc, pools, dwin, gi, eng,
                              wdwg, wdiag[gi], ppool)
            src = out[:mm].rearrange("p (r w) -> p r w", w=WPAD)[:, :, 0:256]
            nc.sync.dma_start(
                qkv_dw[m0:m0 + mm, RB * 256 * b: RB * 256 * (b + 1)], src)
        elif b % 4 == 3:
            eng = dw_eng_of(gi, b // 4)
            out = _dw_compute(nc, tc, pools, dwin, gi, eng,
                              wdwg, wdiag[gi], ppool)
            for k in range(4):
                bb = b - 3 + k
                src = out[32 * k:32 * k + 32].rearrange(
                    "p (r w) -> p r w", w=WPAD)[:, :, 0:256]
                nc.sync.dma_start(
                    qkv_dw[256:288, RB * 256 * bb: RB * 256 * (bb + 1)], src)


def _emit_attn_iter(nc, tc, pools, qv, av, it):
    """One attention iteration: NG=2 h'-row pairs (4 h'-rows)."""
    T = pools['at'].tile([128, NG * 4 * CTOK], BF16, tag="T")
    src = qv[4 * it:4 * it + 4].rearrange(
        "(g dh) (j dw) c -> j g dh dw c", dh=2, dw=2)
    nc.sync.dma_start(
        T[:].rearrange("p (g dh dw c) -> p g dh dw c", g=NG, dh=2, dw=2), src)
    t4 = T[:].rearrange("p (g t c) -> p g t c", g=NG, t=4)
    q = t4[:, :, :, 0:192]
    k = t4[:, :, :, 192:384]
    v = t4[:, :, :, 384:576]

    # QK products: per (g, i) ops, k fully dense, q_i broadcast over j
    # (a single stride-0 operand dim keeps the 2x uop; two broadcasts
    # drop to the slow path)
    P = pools['big'].tile([128, NG * 3072], BF16, tag="big")
    pg = P[:].rearrange("p (g i j hd) -> p g i j hd", g=NG, i=4, j=4)
    for g in range(NG):
        for i in range(4):
            q_b = q[:, g, i].unsqueeze(1).broadcast_to([128, 4, 192])
            nc.vector.tensor_tensor(pg[:, g, i], q_b, k[:, g], op=Mult)

    # logits: reduce over d (24)
    L = pools['sm'].tile([128, NG * 128], F32, tag="L")
    if ATTN_REDUCE_ENG == 'v':
        nc.vector.tensor_reduce(
            L[:], P[:].rearrange("p (s d) -> p s d", d=24), axis=AX, op=Add)
    else:
        # GpSimd has no free-axis reduce; use a tensor_tensor add tree
        ps = P[:].rearrange("p (s d) -> p s d", d=24)
        B8 = pools['sm'].tile([128, NG * 128 * 8], BF16, tag="B8")
        b8 = B8[:].rearrange("p (s d) -> p s d", d=8)
        nc.gpsimd.tensor_tensor(b8, ps[:, :, 0:8], ps[:, :, 8:16], op=Add)
        nc.gpsimd.tensor_tensor(b8, b8, ps[:, :, 16:24], op=Add)
        B2 = pools['sm'].tile([128, NG * 128 * 2], BF16, tag="B2")
        b2 = B2[:].rearrange("p (s d) -> p s d", d=2)
        nc.gpsimd.tensor_tensor(
            b2, b8[:, :, 0:2],
            b8[:, :, 2:4], op=Add)
        nc.gpsimd.tensor_tensor(
            b2, b2, b8[:, :, 4:6], op=Add)
        nc.gpsimd.tensor_tensor(
            b2, b2, b8[:, :, 6:8], op=Add)
        nc.gpsimd.tensor_tensor(
            L[:].rearrange("p (s d) -> p s d", d=1),
            b2[:, :, 0:1], b2[:, :, 1:2], op=Add)

    # exp (no expansion), then fold in 1/S.  (g i) merges to one dim.
    EE = pools['sm'].tile([128, NG * 128], BF16, tag="EE")
    nc.scalar.activation(EE[:], L[:], ExpF, scale=float(SCALE))
    S = pools['sm'].tile([128, NG * 32], F32, tag="S")
    ee3 = EE[:].rearrange("p (gi j h) -> p gi j h", j=4, h=8)
    nc.vector.tensor_reduce(
        S[:].rearrange("p (gi h) -> p gi h", h=8),
        ee3.rearrange("p gi j h -> p gi h j"), axis=AX, op=Add)
    R = pools['sm'].tile([128, NG * 32], F32, tag="R")
    nc.vector.reciprocal(R[:], S[:])
    EN = pools['sm'].tile([128, NG * 128], BF16, tag="EN")
    r_b = R[:].rearrange("p (gi h) -> p gi h", h=8).unsqueeze(
        2).broadcast_to([128, NG * 4, 4, 8])
    nc.gpsimd.tensor_tensor(
        EN[:].rearrange("p (gi j h) -> p gi j h", j=4, h=8),
        ee3, r_b, op=Mult)

    # AV products: expand EN over d, then dense mult with v (bcast over i,
    # per-g since (g i) can't merge on the v side)
    P2 = pools['big'].tile([128, NG * 3072], BF16, tag="big")
    pg2 = P2[:].rearrange("p (g i j hd) -> p g i j hd", g=NG, i=4, j=4)
    EX = pools['big'].tile([128, NG * 3072], BF16, tag="big")
    exv = EX[:].rearrange("p (s d) -> p s d", d=24)
    enb = EN[:].unsqueeze(2).broadcast_to([128, NG * 128, 24])
    if ATTN_EXPAND_ENG == 's':
        nc.scalar.copy(exv, enb)
    else:
        _eng(nc, ATTN_EXPAND_ENG).tensor_copy(exv, enb)
    exg = EX[:].rearrange("p (g i j hd) -> p g i j hd", g=NG, i=4, j=4)
    for g in range(NG):
        for i in range(4):
            _eng(nc, ATTN_AVMULT_ENG).tensor_tensor(
                pg2[:, g, i], exg[:, g, i], v[:, g], op=Mult)

    # sum over j ((g i) merges)
    A = pools['at'].tile([128, NG * 1536], BF16, tag="A")
    a4 = A[:].rearrange("p (gi u hd) -> p gi u hd", u=2, hd=192)
    p24 = P2[:].rearrange("p (gi j hd) -> p gi j hd", j=4, hd=192)
    nc.vector.tensor_tensor(a4, p24[:, :, 0:2, :], p24[:, :, 2:4, :], op=Add)
    U = pools['at'].tile([128, NG * 768], BF16, tag="U")
    u3 = U[:].rearrange("p (gi hd) -> p gi hd", hd=192)
    nc.vector.tensor_tensor(u3, a4[:, :, 0], a4[:, :, 1], op=Add)

    dst = av[4 * it:4 * it + 4].rearrange(
        "(g dh) (j dw) c -> j g dh dw c", dh=2, dw=2)
    nc.scalar.dma_start(
        dst, U[:].rearrange("p (g dh dw c) -> p g dh dw c", g=NG, dh=2, dw=2))


def _emit_proj(nc, tc, pools, ppool, attn_dram, wp, out_dram):
    NT = 1024
    for n in range(NPX // NT):
        xt = pools['pj'].tile([C_ATTN, NT], BF16, tag="pjx")
        nc.sync.dma_start(xt[:], attn_dram[:, bass.ts(n, NT)])
        for mi, (m0, m1) in enumerate([(0, 128), (128, 192)]):
            mm = m1 - m0
            ot = pools['pj'].tile([128, NT], BF16, tag=f"pjo{mi}")
            for s in range(NT // 512):
                ps = ppool.tile([128, 512], F32, tag=f"pj{s % 2}")
                nc.tensor.matmul(ps[:mm], wp[:, m0:m1],
                                 xt[:, bass.ts(s, 512)], start=True, stop=True)
                nc.scalar.copy(ot[:mm, bass.ts(s, 512)], ps[:mm])
            nc.gpsimd.dma_start(out_dram[m0:m1, bass.ts(n, NT)], ot[:mm])


def _build():
    if "nc" in _CACHE:
        return _CACHE["nc"]
    nc = bacc.Bacc("TRN2", target_bir_lowering=False, debug=False,
                   num_devices=8)
    xs = nc.dram_tensor("xs", [C_IN, NPX], BF16, kind="ExternalInput").ap()
    wqkvT = nc.dram_tensor("wqkvT", [C_IN, C_QKV], BF16,
                           kind="ExternalInput").ap()
    wdwg_d = nc.dram_tensor("wdwg", [128, 27], F32, kind="ExternalInput").ap()
    wdiag_d = nc.dram_tensor("wdiag", [3, 9, 128, 128], BF16,
                             kind="ExternalInput").ap()
    wprojT = nc.dram_tensor("wprojT", [C_ATTN, 192], BF16,
                            kind="ExternalInput").ap()
    out = nc.dram_tensor("out", [192, NPX], BF16, kind="ExternalOutput").ap()

    qkv_dw = nc.dram_tensor("qkv_dw_buf", [C_QKV, NPX], BF16).ap()
    attn_dram = nc.dram_tensor("attn_buf", [C_ATTN, NPX], BF16).ap()

    qv = qkv_dw.rearrange("c p -> (c p)").rearrange(
        "(hh ww cc) -> hh ww cc", ww=256, cc=CTOK)
    av = attn_dram.rearrange("c p -> (c p)").rearrange(
        "(hh ww cc) -> hh ww cc", ww=256, cc=192)

    from contextlib import ExitStack
    with tile.TileContext(nc) as tc:
        with ExitStack() as ctx:
            pools = {}
            ppool = ctx.enter_context(
                tc.tile_pool(name="psum", bufs=1, space="PSUM"))
            pools['x'] = ctx.enter_context(tc.tile_pool(name="x", bufs=2))
            pools['dwin'] = ctx.enter_context(
                tc.tile_pool(name="dwin", bufs=2))
            pools['dwacc'] = ctx.enter_context(
                tc.tile_pool(name="dwacc", bufs=2))
            pools['dwy'] = ctx.enter_context(
                tc.tile_pool(name="dwy", bufs=1))
            pools['w'] = ctx.enter_context(tc.tile_pool(name="w", bufs=1))
            pools['at'] = ctx.enter_context(tc.tile_pool(name="at", bufs=2))
            pools['big'] = ctx.enter_context(tc.tile_pool(name="big", bufs=3))
            pools['sm'] = ctx.enter_context(tc.tile_pool(name="sm", bufs=2))
            pools['pj'] = ctx.enter_context(tc.tile_pool(name="pj", bufs=2))

            # weights
            wq0 = pools['w'].tile([128, C_QKV], BF16, tag="wq0")
            wq1 = pools['w'].tile([64, C_QKV], BF16, tag="wq1")
            nc.sync.dma_start(wq0[:], wqkvT[0:128, :])
            nc.sync.dma_start(wq1[:], wqkvT[128:192, :])
            wdwg = pools['w'].tile([128, 27], F32, tag="wdwg")
            nc.sync.dma_start(wdwg[:], wdwg_d)
            wdiag = []
            for gi in range(3):
                wt = pools['w'].tile([128, 9 * 128], BF16, tag=f"wdiag{gi}")
                nc.sync.dma_start(
                    wt[:].rearrange("p (t m) -> p t m", t=9),
                    wdiag_d[gi].rearrange("t k m -> k t m"))
                wdiag.append(wt)
            wp = pools['w'].tile([C_ATTN, 192], BF16, tag="wp")
            nc.sync.dma_start(wp[:], wprojT)

            # pass 1: group 0
            for b in range(NBAND):
                _emit_band(nc, tc, pools, ppool, xs, wq0, wq1, qkv_dw,
                           wdwg, wdiag, b, [0],
                           lambda gi, i: DW_G0[i])
            # pass 2: groups 1+2, with early attention iters interleaved
            early = [i for i in range(NITER) if 4.5 * (2 * i + 2) <= 128.0]
            ei = 0
            for b in range(NBAND):
                _emit_band(nc, tc, pools, ppool, xs, wq0, wq1, qkv_dw,
                           wdwg, wdiag, b, [1, 2],
                           lambda gi, i: DW_G1[i] if gi == 1 else DW_G2[i])
                want = (b + 1) * len(early) // NBAND
                while ei < want:
                    _emit_attn_iter(nc, tc, pools, qv, av, early[ei])
                    ei += 1
            for it in range(len(early), NITER):
                _emit_attn_iter(nc, tc, pools, qv, av, it)
            _emit_proj(nc, tc, pools, ppool, attn_dram, wp, out)
    nc.compile()
    _CACHE["nc"] = nc
    return nc


def kernel(x, w_qkv, w_dw, w_proj, shuffle):
    import ml_dtypes
    bf = ml_dtypes.bfloat16
    x = np.asarray(x, dtype=np.float32)
    w_qkv = np.asarray(w_qkv, dtype=np.float32)
    w_dw = np.asarray(w_dw, dtype=np.float32)
    w_proj = np.asarray(w_proj, dtype=np.float32)
    do_shuffle = bool(int(np.asarray(shuffle)))

    B = x.shape[0]
    xf = x.reshape(B, C_IN, NPX)
    if do_shuffle:
        xf = xf[:, :, _shuffle_perm()]

    wq = w_qkv[:, :, 0, 0]                      # [576, 192]
    wqT = np.ascontiguousarray(wq.T)            # [192, 576]
    wdw_f = w_dw[:, 0].reshape(576, 9)          # [576, 9]
    wpj = w_proj[:, :, 0, 0]                    # [192, 192]

    in_maps = []
    for b in range(B):
        for s in range(2):
            wdw_h = wdw_f[s * C_QKV:(s + 1) * C_QKV]      # [288, 9]
            # group tap weights [128, 27]: group gi cols 9gi..9gi+9,
            # partition p -> channel 128*gi+p (gi<2) or 256+p%32
            wdwg = np.zeros((128, 27), dtype=np.float32)
            wdwg[:, 0:9] = wdw_h[0:128]
            wdwg[:, 9:18] = wdw_h[128:256]
            wdwg[:, 18:27] = wdw_h[256 + (np.arange(128) % 32)]
            wdiag = np.zeros((3, 9, 128, 128), dtype=bf)
            for gi in range(3):
                ch = (np.arange(128) + 128 * gi if gi < 2
                      else 256 + np.arange(128) % 32)
                for t in range(9):
                    wdiag[gi, t][np.arange(128), np.arange(128)] = \
                        wdw_h[ch, t].astype(bf)
            in_maps.append({
                "xs": np.ascontiguousarray(xf[b]).astype(bf),
                "wqkvT": np.ascontiguousarray(
                    wqT[:, s * C_QKV:(s + 1) * C_QKV]).astype(bf),
                "wdwg": wdwg,
                "wdiag": wdiag,
                "wprojT": np.ascontiguousarray(
                    wpj[:, s * C_ATTN:(s + 1) * C_ATTN].T).astype(bf),
            })

    nc = _build()
    res = run_bass_kernel_spmd(nc, in_maps, core_ids=list(range(8)),
                               trace=bool(int(os.environ.get("KERNEL_TRACE", "0"))))
    _CACHE["last_results"] = res

    outs = [res.results[i]["out"] for i in range(8)]
    of = np.stack([outs[2 * b].astype(np.float32) + outs[2 * b + 1].astype(np.float32)
                   for b in range(B)])
    if do_shuffle:
        of = of[:, :, _shuffle_back_perm()]
    return of.reshape(B, 192, 256, 256).astype(np.float32)
